# revision 18
# baseline (speedup 1.0000x reference)
"""Trainium2 Bass kernel for nn_Dense: y = gelu_tanh(fp8qdq(x) @ fp8qdq(W) + b).

Strategy
--------
Host side: quantize x and W to float8_e4m3fn exactly as the reference does
(scale=1 quantize/dequantize), pre-interleave both operands into the SBUF
layout ([partition, chunk, ks, inner]) so every input DMA is a fully
contiguous per-partition copy, and shard 2-D: 4 token-shards x 2
unit-shards across the 8 cores (minimizes per-core input bytes: 1MB x +
2MB W fp8 vs 4.5MB pure data-parallel).

Device side (per core), hand-rolled semaphore pipeline (no TileContext —
saves the tile entry/exit barriers):
  sync   : input DMA triggers (w ni-chunks + xt mi-chunks, need-order),
           then odd-group output DMAs on the same (by then idle) HWDGE ring
  tensor : 8 dummy DoubleRow matmuls to warm the PE HAM clock-gate while
           inputs are in flight, then per group g (column-major, all mi for
           ni=0 first so only w0+xt gate the first 8 groups): 4 DoubleRow
           fp8 matmuls (K=256 each) accumulating one PSUM bank
  scalar : per group: Gelu_apprx_tanh PSUM->SBUF, even-group output DMAs
           on the scalar HWDGE ring
  gpsimd : final semaphore/dma reset so repeat executions stay correct

The fp8 products are exact in f32 accumulation, so the only deviation from
the f32 reference is summation order + the gelu LUT (~7e-5 norm rel err).

TRN's e4m3 (ml_dtypes.float8_e4m3, IEEE-ish, max 240) and the reference's
float8_e4m3fn (OCP, max 448) share bit patterns for |v| <= 240; inputs here
are |v| < ~16 so a byte-level reinterpret is exact.

bias is zero in this problem's setup_inputs; a general Tile-based path with
a broadcast bias add is kept for nonzero bias.
"""

import sys

sys.path.insert(0, "/opt/trn_rl_repo")

from contextlib import ExitStack

import ml_dtypes
import numpy as np

import concourse.bacc as bacc
import concourse.mybir as mybir
from concourse.bass_utils import run_bass_kernel_spmd

N_CORES = 8
TOKENS, D_IN, UNITS = 4096, 1024, 4096

TOK_GRID, UNIT_GRID = 4, 2
TOK_SH = TOKENS // TOK_GRID          # 1024
UNIT_SH = UNITS // UNIT_GRID         # 2048

P = 128
KS = D_IN // P                       # 8 k-subtiles of 128
KP = KS // 2                         # 4 DoubleRow k-pairs (K=256 each)
M_TILES = TOK_SH // P                # 8
NT = 512                             # one PSUM bank of f32
N_TILES = UNIT_SH // NT              # 4
GROUPS = M_TILES * N_TILES           # 32

NB = 6                               # PSUM banks in rotation
OB = GROUPS                          # one SBUF output slot per group (no reuse)
N_WARM = 8                           # dummy matmuls to warm the PE clock gate

_prog_cache = {}


def _build_raw_program():
    """Fast path (zero bias): raw bacc, hand-rolled semaphores."""
    nc = bacc.Bacc("TRN2", target_bir_lowering=False)

    xt = nc.dram_tensor(
        "xt", [P, M_TILES, KS, P], mybir.dt.float8e4, kind="ExternalInput"
    )
    w = nc.dram_tensor(
        "w", [P, N_TILES, KS, NT], mybir.dt.float8e4, kind="ExternalInput"
    )
    y = nc.dram_tensor("y", [TOK_SH, UNIT_SH], mybir.dt.float32, kind="ExternalOutput")

    xt_sb = nc.alloc_sbuf_tensor("xt_sb", [P, M_TILES, KS, P], mybir.dt.float8e4)
    w_sb = nc.alloc_sbuf_tensor("w_sb", [P, N_TILES, KS, NT], mybir.dt.float8e4)
    out_sb = [
        nc.alloc_sbuf_tensor(f"out_sb{i}", [P, NT], mybir.dt.float32)
        for i in range(OB)
    ]
    scratch = nc.alloc_sbuf_tensor("scratch", [P, 8], mybir.dt.float32)
    warm_sb = nc.alloc_sbuf_tensor("warm_sb", [P, 2, NT], mybir.dt.float8e4)
    psum = [
        nc.alloc_psum_tensor(f"ps{b}", [P, NT], mybir.dt.float32) for b in range(NB)
    ]
    ps_warm = nc.alloc_psum_tensor("ps_warm", [P, NT], mybir.dt.float32)

    xt_sems = [nc.alloc_semaphore(f"xt_sem{i}") for i in range(M_TILES)]
    w_sems = [nc.alloc_semaphore(f"w_sem{i}") for i in range(N_TILES)]
    mm_sem = nc.alloc_semaphore("mm_sem")
    gelu_sem = nc.alloc_semaphore("gelu_sem")
    out_semA = nc.alloc_semaphore("out_semA")   # scalar-ring outputs (even g)
    out_semB = nc.alloc_semaphore("out_semB")   # sync-ring outputs (odd g)
    all_sems = xt_sems + w_sems + [mm_sem, gelu_sem, out_semA, out_semB]

    with nc.Block() as block:

        @block.sync
        def _(sync):
            # Contiguous per-partition copies; issue order == need order
            # for the column-major group loop below.
            sync.dma_start(out=w_sb[:, 0, :, :], in_=w[:, 0, :, :]).then_inc(
                w_sems[0], 16
            )
            for mi in range(M_TILES):
                sync.dma_start(
                    out=xt_sb[:, mi, :, :], in_=xt[:, mi, :, :]
                ).then_inc(xt_sems[mi], 16)
            for ni in range(1, N_TILES):
                sync.dma_start(out=w_sb[:, ni, :, :], in_=w[:, ni, :, :]).then_inc(
                    w_sems[ni], 16
                )
            # Odd-group outputs ride the (now idle) sync HWDGE ring so the
            # two rings split the 8MB output stream.
            for g in range(1, GROUPS, 2):
                ni, mi = divmod(g, M_TILES)
                sync.wait_ge(gelu_sem, g + 1)
                sync.dma_start(
                    out=y[mi * P : (mi + 1) * P, ni * NT : (ni + 1) * NT],
                    in_=out_sb[g % OB][:, :],
                ).then_inc(out_semB, 16)
            sync.wait_ge(out_semB, 16 * (GROUPS // 2))

        @block.tensor
        def _(t):
            # Warm the HAM clock gate while input DMAs are in flight.
            for _i in range(N_WARM):
                t.matmul(
                    ps_warm[:, :],
                    lhsT=warm_sb[:, :, 0:P],
                    rhs=warm_sb[:, :, :],
                    start=True,
                    stop=True,
                    perf_mode=mybir.MatmulPerfMode.DoubleRow,
                )
            for g in range(GROUPS):
                ni, mi = divmod(g, M_TILES)
                if mi == 0:
                    t.wait_ge(w_sems[ni], 16)
                if ni == 0:
                    t.wait_ge(xt_sems[mi], 16)
                if g >= NB:
                    t.wait_ge(gelu_sem, g - NB + 1)
                ps = psum[g % NB]
                for kp in range(KP):
                    mm = t.matmul(
                        ps[:, :],
                        lhsT=xt_sb[:, mi, 2 * kp : 2 * kp + 2, :],
                        rhs=w_sb[:, ni, 2 * kp : 2 * kp + 2, :],
                        start=(kp == 0),
                        stop=(kp == KP - 1),
                        perf_mode=mybir.MatmulPerfMode.DoubleRow,
                    )
                mm.then_inc(mm_sem)

        @block.scalar
        def _(s):
            # Dummy activation up front so the Gelu table load overlaps the
            # input DMAs instead of sitting on the first group's drain.
            s.activation(
                scratch[:, :],
                scratch[:, :],
                mybir.ActivationFunctionType.Gelu_apprx_tanh,
            )
            for g in range(GROUPS):
                ni, mi = divmod(g, M_TILES)
                s.wait_ge(mm_sem, g + 1)
                ot = out_sb[g % OB]
                s.activation(
                    ot[:, :],
                    psum[g % NB][:, :],
                    mybir.ActivationFunctionType.Gelu_apprx_tanh,
                ).then_inc(gelu_sem)
                if g % 2 == 0:
                    s.dma_start(
                        out=y[mi * P : (mi + 1) * P, ni * NT : (ni + 1) * NT],
                        in_=ot[:, :],
                    ).then_inc(out_semA, 16)
            s.wait_ge(out_semA, 16 * (GROUPS // 2))

        @block.gpsimd
        def _(gp):
            # Reset semaphores so repeat executions of the loaded NEFF stay
            # correct regardless of runtime re-init behavior.
            gp.wait_ge(out_semA, 16 * (GROUPS // 2))
            gp.wait_ge(out_semB, 16 * (GROUPS // 2))
            nums = sorted(sh.num for sh in all_sems)
            lo, hi = nums[0], nums[-1] + 1
            assert nums == list(range(lo, hi))
            gp.dma_reset(range(lo, hi))
            gp.sem_clear(range(lo, hi))

    nc.compile()
    return nc


def _build_tile_program():
    """General path (nonzero bias): TileContext with broadcast bias add."""
    import concourse.tile as tile

    nc = bacc.Bacc("TRN2", target_bir_lowering=False)

    xt = nc.dram_tensor("xt", [D_IN, TOK_SH], mybir.dt.float8e4, kind="ExternalInput")
    w = nc.dram_tensor("w", [D_IN, UNIT_SH], mybir.dt.float8e4, kind="ExternalInput")
    b = nc.dram_tensor("b", [1, UNIT_SH], mybir.dt.float32, kind="ExternalInput")
    y = nc.dram_tensor("y", [TOK_SH, UNIT_SH], mybir.dt.float32, kind="ExternalOutput")

    with tile.TileContext(nc) as tc, ExitStack() as ctx:
        xt_pool = ctx.enter_context(tc.tile_pool(name="xt", bufs=1))
        w_pool = ctx.enter_context(tc.tile_pool(name="w", bufs=1))
        out_pool = ctx.enter_context(tc.tile_pool(name="out", bufs=8))
        psum_pool = ctx.enter_context(tc.tile_pool(name="psum", bufs=6, space="PSUM"))
        bias_pool = ctx.enter_context(tc.tile_pool(name="bias", bufs=1))
        tmp_pool = ctx.enter_context(tc.tile_pool(name="tmp", bufs=4))

        xt_tile = xt_pool.tile([P, KS, TOK_SH], mybir.dt.float8e4)
        xt_re = xt[:, :].rearrange("(ks p) m -> p ks m", p=P)
        nc.sync.dma_start(xt_tile[:, :, 0:P], xt_re[:, :, 0:P])

        w_tiles = [
            w_pool.tile([P, KS, NT], mybir.dt.float8e4, name=f"w{ni}", tag=f"w{ni}")
            for ni in range(N_TILES)
        ]
        for ni in range(N_TILES):
            nc.sync.dma_start(
                w_tiles[ni][:, :, :],
                w[:, ni * NT : (ni + 1) * NT].rearrange("(ks p) n -> p ks n", p=P),
            )
        for mi in range(1, M_TILES):
            nc.sync.dma_start(
                xt_tile[:, :, mi * P : (mi + 1) * P],
                xt_re[:, :, mi * P : (mi + 1) * P],
            )

        bias_bcast = bias_pool.tile([P, UNIT_SH], mybir.dt.float32)
        nc.sync.dma_start(bias_bcast[:, :], b[0, :].partition_broadcast(P))

        for mi in range(M_TILES):
            for ni in range(N_TILES):
                ps = psum_pool.tile([P, NT], mybir.dt.float32)
                for kp in range(KP):
                    nc.tensor.matmul(
                        ps[:, :],
                        lhsT=xt_tile[:, 2 * kp : 2 * kp + 2, mi * P : (mi + 1) * P],
                        rhs=w_tiles[ni][:, 2 * kp : 2 * kp + 2, :],
                        start=(kp == 0),
                        stop=(kp == KP - 1),
                        perf_mode=mybir.MatmulPerfMode.DoubleRow,
                    )
                ot = out_pool.tile([P, NT], mybir.dt.float32)
                tmp = tmp_pool.tile([P, NT], mybir.dt.float32)
                nc.vector.tensor_add(
                    tmp[:, :], ps[:, :], bias_bcast[:, ni * NT : (ni + 1) * NT]
                )
                nc.scalar.activation(
                    ot[:, :],
                    tmp[:, :],
                    mybir.ActivationFunctionType.Gelu_apprx_tanh,
                )
                nc.sync.dma_start(
                    y[mi * P : (mi + 1) * P, ni * NT : (ni + 1) * NT], ot[:, :]
                )
    nc.compile()
    return nc


def _get_program(with_bias: bool):
    if with_bias not in _prog_cache:
        _prog_cache[with_bias] = (
            _build_tile_program() if with_bias else _build_raw_program()
        )
    return _prog_cache[with_bias]


def _quantize(x, kernel):
    # fp8 quantize on host with reference (OCP e4m3fn) semantics; bytes are
    # reinterpreted as the TRN-compatible ml_dtypes.float8_e4m3 later.
    xq = np.asarray(x, np.float32).astype(ml_dtypes.float8_e4m3fn)
    wq = np.asarray(kernel, np.float32).astype(ml_dtypes.float8_e4m3fn)
    return xq.view(np.uint8), wq.view(np.uint8)


def _run(x, kernel, bias, trace=False):
    assert x.shape == (TOKENS, D_IN) and kernel.shape == (D_IN, UNITS)
    xq_bits, wq_bits = _quantize(x, kernel)
    bf = np.asarray(bias, np.float32).reshape(UNITS)
    with_bias = bool(np.any(bf != 0))
    nc = _get_program(with_bias)

    in_maps = []
    for c in range(N_CORES):
        tg, ug = divmod(c, UNIT_GRID)
        xs = xq_bits[tg * TOK_SH : (tg + 1) * TOK_SH, :]       # [1024, 1024]
        ws = wq_bits[:, ug * UNIT_SH : (ug + 1) * UNIT_SH]     # [1024, 2048]
        if with_bias:
            in_map = {
                "xt": np.ascontiguousarray(xs.T).view(ml_dtypes.float8_e4m3),
                "w": np.ascontiguousarray(ws).view(ml_dtypes.float8_e4m3),
                "b": np.ascontiguousarray(
                    bf[ug * UNIT_SH : (ug + 1) * UNIT_SH].reshape(1, UNIT_SH)
                ),
            }
        else:
            # Pre-interleave into [partition, chunk, ks, inner] DMA layouts.
            # xt_host[p, mi, ks, m] = X[mi*128+m, ks*128+p]
            xt_host = np.ascontiguousarray(
                xs.reshape(M_TILES, P, KS, P).transpose(3, 0, 2, 1)
            )
            # w_host[p, ni, ks, n] = W[ks*128+p, ni*512+n]
            w_host = np.ascontiguousarray(
                ws.reshape(KS, P, N_TILES, NT).transpose(1, 2, 0, 3)
            )
            in_map = {
                "xt": xt_host.view(ml_dtypes.float8_e4m3),
                "w": w_host.view(ml_dtypes.float8_e4m3),
            }
        in_maps.append(in_map)

    res = run_bass_kernel_spmd(nc, in_maps, list(range(N_CORES)), trace=trace)

    out = np.empty((TOKENS, UNITS), np.float32)
    for c in range(N_CORES):
        tg, ug = divmod(c, UNIT_GRID)
        out[tg * TOK_SH : (tg + 1) * TOK_SH, ug * UNIT_SH : (ug + 1) * UNIT_SH] = (
            res.results[c]["y"]
        )
    return out, res


def kernel(x: np.ndarray, kernel: np.ndarray, bias: np.ndarray) -> np.ndarray:
    return _run(x, kernel, bias)[0]


def _ensure_ntff_hook():
    """The agent image's antenv lacks axon_hooks; shim it so trace=True works."""
    try:
        from antenv.axon_hooks import get_axon_ntff_profile_hook  # noqa: F401

        return
    except ImportError:
        pass
    import types

    import antenv

    mod = types.ModuleType("antenv.axon_hooks")
    mod._hook = None

    def set_axon_ntff_profile_hook(h):
        mod._hook = h

    def get_axon_ntff_profile_hook():
        return mod._hook

    mod.set_axon_ntff_profile_hook = set_axon_ntff_profile_hook
    mod.get_axon_ntff_profile_hook = get_axon_ntff_profile_hook
    sys.modules["antenv.axon_hooks"] = mod
    antenv.axon_hooks = mod
    if "/root/.axon_site" not in sys.path:
        sys.path.insert(0, "/root/.axon_site")
    from trn_agent_boot.trn_boot import _ntff_profile_via_ctypes

    set_axon_ntff_profile_hook(
        _ntff_profile_via_ctypes("/opt/axon/libaxon_pjrt.so")
    )


def profile_run(np_inputs):
    """Run with NTFF tracing; returns exec_time_ns (max across traced cores)."""
    _ensure_ntff_hook()
    _, res = _run(
        np_inputs["x"], np_inputs["kernel"], np_inputs["bias"], trace=True
    )
    return res.exec_time_ns
